# revision 23
# baseline (speedup 1.0000x reference)
"""Multi-head attention forward (B=4, L=2048, E=1024, H=16) on 8 NeuronCores.

Sharding: core c handles batch b = c // 2 and head-group g = c % 2 (8 heads,
512 embed dims). Each core computes its QKV projections, attention, and a
partial out-projection over its 512 contraction dims; the host sums the two
partials per batch and adds the bias.

All transposes and bf16 casts happen on the host: each core receives
xqT/xkT/xvT as [E, L] bf16, wqkvT as [E, 3*FG] bf16 (cols q|k|v) and
woutT as [FG, E] bf16. On-chip work is a single fused pipeline:
k-proj -> per (p, lg): q-proj, scores (PE), exp (ACT), AV (PE), with
v-proj and the out-projection interleaved into the PE stream so the
tensor engine never waits on the ACT-bound softmax.

Self-contained: only needs numpy + the concourse stack at /opt/trn_rl_repo.
"""

import os
import sys

import numpy as np

sys.path.insert(0, "/opt/trn_rl_repo")

import ml_dtypes  # noqa: E402

import concourse.bass as bass  # noqa: E402
import concourse.tile as tile  # noqa: E402
from concourse import bacc, mybir  # noqa: E402
from concourse import bass_utils  # noqa: E402

F32 = mybir.dt.float32
BF16 = mybir.dt.bfloat16
EXP = mybir.ActivationFunctionType.Exp
NP_BF16 = ml_dtypes.bfloat16

P = 128          # partitions
L = 2048         # sequence length
E = 1024         # embed dim
FG = 512         # per-core feature slice (8 heads x 64)
D = 64           # head dim
EC = E // P      # 8 e-chunks (contraction tiles for projections)
SC = L // P      # 16 s-chunks
LG = L // 512    # 4 q-windows of 512
FT = FG // P     # 4 head pairs
GRP = 3          # score psum banks per exp group
NU = 2 * SC      # 32 (sc, head) units per (p, lg)


def _build():
    nc = bacc.Bacc("TRN2", target_bir_lowering=False, debug=False, num_devices=8)

    debug = bool(os.environ.get("MHA_DEBUG"))
    xqT_d = nc.dram_tensor("xqT", [E, L], BF16, kind="ExternalInput")
    xkT_d = nc.dram_tensor("xkT", [E, L], BF16, kind="ExternalInput")
    xvT_d = nc.dram_tensor("xvT", [E, L], BF16, kind="ExternalInput")
    wqkvT_d = nc.dram_tensor("wqkvT", [E, 3 * FG], BF16, kind="ExternalInput")
    woutT_d = nc.dram_tensor("woutT", [FG, E], BF16, kind="ExternalInput")
    out_d = nc.dram_tensor("out", [L, E], F32, kind="ExternalOutput")
    if debug:
        dbg_q = nc.dram_tensor("dbg_q", [P, L], BF16, kind="ExternalOutput")
        dbg_k = nc.dram_tensor("dbg_k", [P, L], BF16, kind="ExternalOutput")
        dbg_v = nc.dram_tensor("dbg_v", [P, 1024], BF16, kind="ExternalOutput")
        dbg_a = nc.dram_tensor("dbg_a", [P, 1536], BF16, kind="ExternalOutput")
        dbg_s = nc.dram_tensor("dbg_s", [P, 512], F32, kind="ExternalOutput")
        dbg_n = nc.dram_tensor("dbg_n", [P, L], BF16, kind="ExternalOutput")

    with tile.TileContext(nc) as tc:
        with (
            tc.tile_pool(name="const", bufs=1) as constp,
            tc.tile_pool(name="pers", bufs=1) as pers,
            tc.tile_pool(name="xin", bufs=1) as xin,
            tc.tile_pool(name="xv", bufs=2) as xvp,
            tc.tile_pool(name="xq", bufs=2) as xqp,
            tc.tile_pool(name="stage", bufs=2) as stage,
            tc.tile_pool(name="ps", bufs=2, space="PSUM") as psp,
            tc.tile_pool(name="psav", bufs=1, space="PSUM") as psav,
        ):
            # engine warm-ups (prime DVE cast path + preload the EXP table)
            warm32 = constp.tile([P, 16], F32, tag="warm32", name="warm32")
            nc.vector.memset(warm32[:], 0.0)
            warm16 = constp.tile([P, 16], BF16, tag="warm16", name="warm16")
            nc.vector.tensor_copy(warm16[:], warm32[:])
            warmE = constp.tile([P, 16], BF16, tag="warmE", name="warmE")
            nc.scalar.activation(warmE[:], warm32[:], EXP, scale=0.125)
            warmG = constp.tile([P, 16], F32, tag="warmG", name="warmG")
            nc.gpsimd.memset(warmG[:], 0.0)
            # reciprocal staging: rows 0/32 hold denominators per tail, the
            # rest stays 1.0 so the batched [64,512] reciprocal is stable
            rr = constp.tile([P, 512], F32, tag="rr", name="rr")
            nc.vector.memset(rr[:], 1.0)

            # persistent activations / weights
            kT = [pers.tile([P, L], BF16, tag=f"kT{p}", name=f"kT{p}")
                  for p in range(FT)]
            qT = [pers.tile([P, L], BF16, tag=f"qT{p}", name=f"qT{p}")
                  for p in range(FT)]
            avN = [pers.tile([P, L], BF16, tag=f"avN{p}", name=f"avN{p}")
                   for p in range(FT)]
            # AV stationary tiles: per s-chunk, 4 pairs x 256 cols:
            #   [v_h0(64) | ones(1) | junk(63)]  -> av rows 0:64, sum row 64
            #   [junk(32) | ones(1) | junk(31) | v_h1(64)] -> rows 64:128, sum row 32
            # (junk columns only feed av rows that are never read)
            vst = [pers.tile([P, 1024], BF16, tag=f"vst{s}", name=f"vst{s}")
                   for s in range(SC)]
            wq_sb = [pers.tile([P, 3 * FG], BF16, tag=f"wq{ec}", name=f"wq{ec}")
                     for ec in range(EC)]
            wo_sb = [pers.tile([P, E], BF16, tag=f"wo{e}", name=f"wo{e}")
                     for e in range(FT)]

            # ---- input DMAs, ordered by first use (k-proj path first) ----
            for ec in range(EC):
                nc.sync.dma_start(wq_sb[ec][:, FG:2 * FG],
                                  wqkvT_d.ap()[ec * P:(ec + 1) * P, FG:2 * FG])
            xk = [xin.tile([P, L], BF16, tag=f"x{ec}", name=f"xk{ec}")
                  for ec in range(EC)]
            for ec in range(EC):
                nc.sync.dma_start(xk[ec][:], xkT_d.ap()[ec * P:(ec + 1) * P, :])

            # AV stationary pattern: zero the non-v columns, ones at the
            # denominator columns (64, 160 of each 256-block)
            for s in range(SC):
                t = vst[s]
                nc.vector._memset_packed(
                    bass.AP(t.tensor, t.offset + 64,
                            [[1024, 128], [256, 4], [1, 128]]), 0)
                one = np.float32(1.0).astype(NP_BF16).view(np.uint16)
                nc.vector._memset_packed(
                    bass.AP(t.tensor, t.offset + 64, [[1024, 128], [256, 4]]),
                    int(one))
                nc.vector._memset_packed(
                    bass.AP(t.tensor, t.offset + 160, [[1024, 128], [256, 4]]),
                    int(one))

            def dma_xq_window(lg):
                tiles = [xqp.tile([P, 512], BF16, tag=f"q{ec}", name=f"xq{ec}")
                         for ec in range(EC)]
                for ec in range(EC):
                    nc.sync.dma_start(
                        tiles[ec][:],
                        xqT_d.ap()[ec * P:(ec + 1) * P, lg * 512:(lg + 1) * 512])
                return tiles

            for ec in range(EC):
                nc.sync.dma_start(wq_sb[ec][:, 0:FG],
                                  wqkvT_d.ap()[ec * P:(ec + 1) * P, 0:FG])
            xq_w = dma_xq_window(0)

            def dma_xv_window(w):
                tiles = [xvp.tile([P, 512], BF16, tag=f"v{ec}", name=f"xv{ec}")
                         for ec in range(EC)]
                for ec in range(EC):
                    nc.sync.dma_start(
                        tiles[ec][:],
                        xvT_d.ap()[ec * P:(ec + 1) * P, w * 512:(w + 1) * 512])
                return tiles

            for ec in range(EC):
                nc.sync.dma_start(wq_sb[ec][:, 2 * FG:3 * FG],
                                  wqkvT_d.ap()[ec * P:(ec + 1) * P, 2 * FG:3 * FG])
            xvw = {0: dma_xv_window(0), 1: dma_xv_window(1)}
            for e in range(FT):
                nc.sync.dma_start(wo_sb[e][:], woutT_d.ap()[e * P:(e + 1) * P, :])

            # ---- helpers ----
            def kproj(p):
                # kT[p][:, :] over 4 token windows; 3+1 windows per psum tile
                for w0, nw in ((0, 3), (3, 1)):
                    ps = psp.tile([P, 1536], F32, tag="sc", name="kps")
                    for ec in range(EC):
                        for w in range(nw):
                            nc.tensor.matmul(
                                ps[:, w * 512:(w + 1) * 512],
                                wq_sb[ec][:, FG + p * P:FG + (p + 1) * P],
                                xk[ec][:, (w0 + w) * 512:(w0 + w + 1) * 512],
                                start=(ec == 0), stop=(ec == EC - 1))
                    nc.vector.tensor_copy(
                        kT[p][:, w0 * 512:(w0 + nw) * 512], ps[:, 0:nw * 512])

            def qproj(p, lg):
                # rides the psav rotation so it does not contend with the
                # scores-psum slots at iteration boundaries
                ps = psav.tile([P, 512], F32, tag="avA", name="qps")
                for ec in range(EC):
                    nc.tensor.matmul(
                        ps[:],
                        wq_sb[ec][:, p * P:(p + 1) * P],
                        xq_w[ec][:],
                        start=(ec == 0), stop=(ec == EC - 1))
                nc.vector.tensor_copy(
                    qT[p][:, lg * 512:(lg + 1) * 512], ps[:])

            def vproj(lt, xv_w):
                ps = psp.tile([P, 1536], F32, tag="sc", name="vps")
                for ec in range(EC):
                    nc.tensor.matmul(
                        ps[:, 0:512],
                        xv_w[ec][:, (lt % 4) * P:(lt % 4 + 1) * P],
                        wq_sb[ec][:, 2 * FG:3 * FG],
                        start=(ec == 0), stop=(ec == EC - 1))
                # strided drains: h0 dims -> cols {0:64}+256p, h1 -> {192:256}+256p
                dst0 = bass.AP(vst[lt].tensor, vst[lt].offset,
                               [[1024, 128], [256, 4], [1, 64]])
                src0 = bass.AP(ps.tensor, ps.offset,
                               [[1536, 128], [128, 4], [1, 64]])
                nc.vector.tensor_copy(dst0, src0)
                dst1 = bass.AP(vst[lt].tensor, vst[lt].offset + 192,
                               [[1024, 128], [256, 4], [1, 64]])
                src1 = bass.AP(ps.tensor, ps.offset + 64,
                               [[1536, 128], [128, 4], [1, 64]])
                nc.vector.tensor_copy(dst1, src1)

            def outproj(lg, lt):
                t0 = lg * 512 + lt * P
                ps = psp.tile([P, 1536], F32, tag="sc", name="ops")
                for ec in range(FT):
                    nc.tensor.matmul(
                        ps[:, 0:512], avN[ec][:, t0:t0 + P],
                        wo_sb[ec][:, 0:512],
                        start=(ec == 0), stop=(ec == FT - 1))
                    nc.tensor.matmul(
                        ps[:, 512:1024], avN[ec][:, t0:t0 + P],
                        wo_sb[ec][:, 512:1024],
                        start=(ec == 0), stop=(ec == FT - 1))
                osb = stage.tile([P, E], F32, tag="osb", name="osb", bufs=2)
                nc.scalar.copy(osb[:], ps[:, 0:1024])
                nc.sync.dma_start(out_d.ap()[t0:t0 + P, :], osb[:])

            pending_tails = []

            def attention_iter(p, lg, hook=None):
                avA = psav.tile([P, 512], F32, tag="avA", name="avA")
                avB = psav.tile([P, 512], F32, tag="avB", name="avB")
                av_bank = (avA, avB)

                def av_mms(t0, n, aT, p=p, av_bank=av_bank):
                    for j in range(n):
                        sc, h = divmod(t0 + j, 2)
                        nc.tensor.matmul(
                            av_bank[h][:],
                            vst[sc][:, p * 256 + 128 * h:p * 256 + 128 * h + 128],
                            aT[:, j * 512:(j + 1) * 512],
                            start=(sc == 0), stop=(sc == SC - 1))

                pending = None
                for gi, t0 in enumerate(range(0, NU, GRP)):
                    n = min(GRP, NU - t0)
                    ps = psp.tile([P, 1536], F32, tag="sc", name="scp")
                    for j in range(n):
                        sc, h = divmod(t0 + j, 2)
                        nc.tensor.matmul(
                            ps[:, j * 512:(j + 1) * 512],
                            kT[p][64 * h:64 * h + 64, sc * P:(sc + 1) * P],
                            qT[p][64 * h:64 * h + 64, lg * 512:(lg + 1) * 512],
                            start=True, stop=True)
                    aT = stage.tile([P, 1536], BF16, tag="aT", name="aT", bufs=3)
                    nc.scalar.activation(aT[:, 0:512 * n], ps[:, 0:512 * n],
                                         EXP, scale=0.125)
                    if debug and p == 0 and lg == 0 and gi == 0:
                        nc.sync.dma_start(dbg_a.ap(), aT[:])
                    if gi == 0 and pending_tails:
                        pending_tails.pop()()  # prev iter's tail after this
                        # iter's first exp is queued
                    if hook is not None:
                        hook(gi)
                    if pending is not None:
                        av_mms(*pending)
                    pending = (t0, n, aT)

                def tail(avA=avA, avB=avB, p=p, lg=lg, pending=pending,
                         av_mms=av_mms):
                    av_mms(*pending)
                    avS0 = stage.tile([P, 512], F32, tag="avS0", name="avS0",
                                      bufs=1)
                    nc.vector.tensor_copy(avS0[:], avA[:])
                    avS1 = stage.tile([P, 512], F32, tag="avS1", name="avS1",
                                      bufs=1)
                    nc.vector.tensor_copy(avS1[:], avB[:])
                    # both denominators on partitions 0/32 -> one reciprocal
                    nc.vector.tensor_copy(rr[0:1, :], avS0[64:65, :])
                    nc.vector.tensor_copy(rr[32:33, :], avS1[32:33, :])
                    nc.vector.reciprocal(rr[0:64, :], rr[0:64, :])
                    r1 = stage.tile([1, 512], F32, tag="r1", name="r1", bufs=1)
                    nc.vector.tensor_copy(r1[0:1, :], rr[32:33, :])
                    bc0 = stage.tile([P, 512], F32, tag="bc0", name="bc0",
                                     bufs=1)
                    nc.gpsimd.partition_broadcast(bc0[:], rr[0:1, :])
                    bc1 = stage.tile([P, 512], F32, tag="bc1", name="bc1",
                                     bufs=1)
                    nc.gpsimd.partition_broadcast(bc1[:], r1[0:1, :])
                    nc.vector.tensor_mul(
                        avN[p][0:64, lg * 512:(lg + 1) * 512],
                        avS0[0:64, :], bc0[0:64, :])
                    nc.vector.tensor_mul(
                        avN[p][64:128, lg * 512:(lg + 1) * 512],
                        avS1[64:128, :], bc1[64:128, :])
                    if debug and p == 0 and lg == 0:
                        nc.sync.dma_start(dbg_s.ap(), avS0[:])
                pending_tails.append(tail)

            # ---- the fused schedule ----
            def vproj_hook(gi):
                if gi < 8:
                    if gi in (2, 4):
                        xvw[gi // 2 + 1] = dma_xv_window(gi // 2 + 1)
                    w = gi // 2
                    vproj(2 * gi, xvw[w])
                    vproj(2 * gi + 1, xvw[w])

            for lg in range(LG):
                for p in range(FT):
                    if lg == 0:
                        kproj(p)
                    qproj(p, lg)
                    attention_iter(p, lg, hook=vproj_hook if (p == 0 and lg == 0)
                                   else None)
                    if lg > 0:
                        outproj(lg - 1, p)
                if lg < LG - 1:
                    xq_w = dma_xq_window(lg + 1)
            while pending_tails:
                pending_tails.pop()()
            for lt in range(FT):
                outproj(LG - 1, lt)
            if debug:
                nc.sync.dma_start(dbg_q.ap(), qT[0][:])
                nc.sync.dma_start(dbg_k.ap(), kT[0][:])
                nc.sync.dma_start(dbg_v.ap(), vst[0][:])
                nc.sync.dma_start(dbg_n.ap(), avN[0][:])

    nc.compile()
    return nc


_NC = None


def _get_nc():
    global _NC
    if _NC is None:
        _NC = _build()
    return _NC


def _shard_inputs(query, key, value, in_proj_weight, out_proj_weight):
    B = query.shape[0]
    # per-batch transposed bf16 activations (shared by the 2 cores per batch)
    xT = {}
    for b in range(B):
        xT[b] = tuple(
            np.ascontiguousarray(np.asarray(x[b], dtype=np.float32).T).astype(NP_BF16)
            for x in (query, key, value))
    # per-head-group weight blocks
    wblk = {}
    for g in range(2):
        sl = slice(FG * g, FG * g + FG)
        wq = in_proj_weight[0 * E:1 * E][sl]
        wk = in_proj_weight[1 * E:2 * E][sl]
        wv = in_proj_weight[2 * E:3 * E][sl]
        wqkvT = np.ascontiguousarray(
            np.concatenate([wq.T, wk.T, wv.T], axis=1)).astype(NP_BF16)
        woutT = np.ascontiguousarray(out_proj_weight[:, sl].T).astype(NP_BF16)
        wblk[g] = (wqkvT, woutT)
    in_maps = []
    for c in range(8):
        b, g = divmod(c, 2)
        xq, xk, xv = xT[b]
        wqkvT, woutT = wblk[g]
        in_maps.append({
            "xqT": xq, "xkT": xk, "xvT": xv,
            "wqkvT": wqkvT, "woutT": woutT,
        })
    return in_maps


def run_sharded(in_maps, **kwargs):
    nc = _get_nc()
    return bass_utils.run_bass_kernel_spmd(
        nc, in_maps, core_ids=list(range(8)), **kwargs)


def kernel(query, key, value, in_proj_weight, out_proj_weight, out_proj_bias):
    query = np.asarray(query, dtype=np.float32)
    key = np.asarray(key, dtype=np.float32)
    value = np.asarray(value, dtype=np.float32)
    in_proj_weight = np.asarray(in_proj_weight, dtype=np.float32)
    out_proj_weight = np.asarray(out_proj_weight, dtype=np.float32)
    out_proj_bias = np.asarray(out_proj_bias, dtype=np.float32)

    in_maps = _shard_inputs(query, key, value, in_proj_weight, out_proj_weight)
    res = run_sharded(in_maps)
    out = np.empty((4, L, E), dtype=np.float32)
    for b in range(4):
        out[b] = res.results[2 * b]["out"] + res.results[2 * b + 1]["out"]
    out += out_proj_bias
    return out


# revision 25
# speedup vs baseline: 1.1799x; 1.1799x over previous
"""Multi-head attention forward (B=4, L=2048, E=1024, H=16) on 8 NeuronCores.

Sharding: core c handles batch b = c // 2 and head-group g = c % 2 (8 heads,
512 embed dims). Each core computes its QKV projections, attention, and a
partial out-projection over its 512 contraction dims; the host sums the two
partials per batch and adds the bias.

All transposes and bf16 casts happen on the host: each core receives
xqT/xkT/xvT as [E, L] bf16, wqkvT as [E, 3*FG] bf16 (cols q|k|v) and
woutT as [FG, E] bf16. On-chip work is a single fused pipeline:
k-proj -> per (p, lg): q-proj, scores (PE), exp (ACT), AV (PE), with
v-proj and the out-projection interleaved into the PE stream so the
tensor engine never waits on the ACT-bound softmax.

Self-contained: only needs numpy + the concourse stack at /opt/trn_rl_repo.
"""

import os
import sys

import numpy as np

sys.path.insert(0, "/opt/trn_rl_repo")

import ml_dtypes  # noqa: E402

import concourse.bass as bass  # noqa: E402
import concourse.tile as tile  # noqa: E402
from concourse import bacc, mybir  # noqa: E402
from concourse import bass_utils  # noqa: E402

F32 = mybir.dt.float32
BF16 = mybir.dt.bfloat16
EXP = mybir.ActivationFunctionType.Exp
NP_BF16 = ml_dtypes.bfloat16

P = 128          # partitions
L = 2048         # sequence length
E = 1024         # embed dim
FG = 512         # per-core feature slice (8 heads x 64)
D = 64           # head dim
EC = E // P      # 8 e-chunks (contraction tiles for projections)
SC = L // P      # 16 s-chunks
LG = L // 512    # 4 q-windows of 512
FT = FG // P     # 4 head pairs
GRP = 3          # score psum banks per exp group
NU = 2 * SC      # 32 (sc, head) units per (p, lg)


def _build():
    nc = bacc.Bacc("TRN2", target_bir_lowering=False, debug=False, num_devices=8)

    debug = bool(os.environ.get("MHA_DEBUG"))
    xqT_d = nc.dram_tensor("xqT", [E, L], BF16, kind="ExternalInput")
    xkT_d = nc.dram_tensor("xkT", [E, L], BF16, kind="ExternalInput")
    xvT_d = nc.dram_tensor("xvT", [E, L], BF16, kind="ExternalInput")
    wqkvT_d = nc.dram_tensor("wqkvT", [E, 3 * FG], BF16, kind="ExternalInput")
    woutT_d = nc.dram_tensor("woutT", [FG, E], BF16, kind="ExternalInput")
    out_d = nc.dram_tensor("out", [L, E], F32, kind="ExternalOutput")
    if debug:
        dbg_q = nc.dram_tensor("dbg_q", [P, L], BF16, kind="ExternalOutput")
        dbg_k = nc.dram_tensor("dbg_k", [P, L], BF16, kind="ExternalOutput")
        dbg_v = nc.dram_tensor("dbg_v", [P, 1024], BF16, kind="ExternalOutput")
        dbg_a = nc.dram_tensor("dbg_a", [P, 1536], BF16, kind="ExternalOutput")
        dbg_s = nc.dram_tensor("dbg_s", [P, 512], F32, kind="ExternalOutput")
        dbg_n = nc.dram_tensor("dbg_n", [P, L], BF16, kind="ExternalOutput")

    with tile.TileContext(nc) as tc:
        with (
            tc.tile_pool(name="const", bufs=1) as constp,
            tc.tile_pool(name="pers", bufs=1) as pers,
            tc.tile_pool(name="xin", bufs=1) as xin,
            tc.tile_pool(name="xv", bufs=2) as xvp,
            tc.tile_pool(name="xq", bufs=2) as xqp,
            tc.tile_pool(name="stage", bufs=2) as stage,
            tc.tile_pool(name="ps", bufs=2, space="PSUM") as psp,
            tc.tile_pool(name="psav", bufs=1, space="PSUM") as psav,
        ):
            # engine warm-ups (prime DVE cast path + preload the EXP table)
            warm32 = constp.tile([P, 16], F32, tag="warm32", name="warm32")
            nc.vector.memset(warm32[:], 0.0)
            warm16 = constp.tile([P, 16], BF16, tag="warm16", name="warm16")
            nc.vector.tensor_copy(warm16[:], warm32[:])
            warmE = constp.tile([P, 16], BF16, tag="warmE", name="warmE")
            nc.scalar.activation(warmE[:], warm32[:], EXP, scale=0.125)
            warmG = constp.tile([P, 16], F32, tag="warmG", name="warmG")
            nc.gpsimd.memset(warmG[:], 0.0)
            # reciprocal staging: rows 0/32 hold denominators per tail, the
            # rest stays 1.0 so the batched [64,512] reciprocal is stable
            rr = constp.tile([P, 512], F32, tag="rr", name="rr")
            nc.vector.memset(rr[:], 1.0)

            # persistent activations / weights
            kT = [pers.tile([P, L], BF16, tag=f"kT{p}", name=f"kT{p}")
                  for p in range(FT)]
            qT = [pers.tile([P, L], BF16, tag=f"qT{p}", name=f"qT{p}")
                  for p in range(FT)]
            avN = [pers.tile([P, L], BF16, tag=f"avN{p}", name=f"avN{p}")
                   for p in range(FT)]
            # AV stationary tiles: per s-chunk, 4 pairs x 256 cols:
            #   [v_h0(64) | ones(1) | junk(63)]  -> av rows 0:64, sum row 64
            #   [junk(32) | ones(1) | junk(31) | v_h1(64)] -> rows 64:128, sum row 32
            # (junk columns only feed av rows that are never read)
            vst = [pers.tile([P, 1024], BF16, tag=f"vst{s}", name=f"vst{s}")
                   for s in range(SC)]
            wq_sb = [pers.tile([P, 3 * FG], BF16, tag=f"wq{ec}", name=f"wq{ec}")
                     for ec in range(EC)]
            wo_sb = [pers.tile([P, E], BF16, tag=f"wo{e}", name=f"wo{e}")
                     for e in range(FT)]

            # ---- input DMAs, ordered by first use (k-proj path first) ----
            for ec in range(EC):
                nc.sync.dma_start(wq_sb[ec][:, FG:2 * FG],
                                  wqkvT_d.ap()[ec * P:(ec + 1) * P, FG:2 * FG])
            xk = [xin.tile([P, L], BF16, tag=f"x{ec}", name=f"xk{ec}")
                  for ec in range(EC)]
            for ec in range(EC):
                nc.sync.dma_start(xk[ec][:], xkT_d.ap()[ec * P:(ec + 1) * P, :])

            # AV stationary pattern: zero the non-v columns, ones at the
            # denominator columns (64, 160 of each 256-block)
            one = int(np.float32(1.0).astype(NP_BF16).view(np.uint16))
            for s in range(SC):
                t = vst[s]
                nc.gpsimd._memset_packed(
                    bass.AP(t.tensor, t.offset + 64,
                            [[1024, 128], [256, 4], [1, 128]]), 0)
                nc.gpsimd._memset_packed(
                    bass.AP(t.tensor, t.offset + 64, [[1024, 128], [256, 4]]),
                    one)
                nc.gpsimd._memset_packed(
                    bass.AP(t.tensor, t.offset + 160, [[1024, 128], [256, 4]]),
                    one)

            def dma_xq_window(lg):
                tiles = [xqp.tile([P, 512], BF16, tag=f"q{ec}", name=f"xq{ec}")
                         for ec in range(EC)]
                for ec in range(EC):
                    nc.sync.dma_start(
                        tiles[ec][:],
                        xqT_d.ap()[ec * P:(ec + 1) * P, lg * 512:(lg + 1) * 512])
                return tiles

            for ec in range(EC):
                nc.sync.dma_start(wq_sb[ec][:, 0:FG],
                                  wqkvT_d.ap()[ec * P:(ec + 1) * P, 0:FG])
            xq_w = dma_xq_window(0)

            def dma_xv_window(w):
                tiles = [xvp.tile([P, 512], BF16, tag=f"v{ec}", name=f"xv{ec}")
                         for ec in range(EC)]
                for ec in range(EC):
                    nc.sync.dma_start(
                        tiles[ec][:],
                        xvT_d.ap()[ec * P:(ec + 1) * P, w * 512:(w + 1) * 512])
                return tiles

            for ec in range(EC):
                nc.sync.dma_start(wq_sb[ec][:, 2 * FG:3 * FG],
                                  wqkvT_d.ap()[ec * P:(ec + 1) * P, 2 * FG:3 * FG])
            xvw = {0: dma_xv_window(0), 1: dma_xv_window(1)}
            for e in range(FT):
                nc.sync.dma_start(wo_sb[e][:], woutT_d.ap()[e * P:(e + 1) * P, :])

            # ---- helpers ----
            def kproj(p):
                # kT[p][:, :] over 4 token windows; 3+1 windows per psum tile
                for w0, nw in ((0, 3), (3, 1)):
                    ps = psp.tile([P, 1536], F32, tag="sc", name="kps")
                    for ec in range(EC):
                        for w in range(nw):
                            nc.tensor.matmul(
                                ps[:, w * 512:(w + 1) * 512],
                                wq_sb[ec][:, FG + p * P:FG + (p + 1) * P],
                                xk[ec][:, (w0 + w) * 512:(w0 + w + 1) * 512],
                                start=(ec == 0), stop=(ec == EC - 1))
                    nc.vector.tensor_copy(
                        kT[p][:, w0 * 512:(w0 + nw) * 512], ps[:, 0:nw * 512])

            def qproj(p, lg):
                ps = psp.tile([P, 1536], F32, tag="sc", name="qps")
                for ec in range(EC):
                    nc.tensor.matmul(
                        ps[:, 0:512],
                        wq_sb[ec][:, p * P:(p + 1) * P],
                        xq_w[ec][:],
                        start=(ec == 0), stop=(ec == EC - 1))
                nc.vector.tensor_copy(
                    qT[p][:, lg * 512:(lg + 1) * 512], ps[:, 0:512])

            def vproj(lt, xv_w):
                ps = psp.tile([P, 1536], F32, tag="sc", name="vps")
                for ec in range(EC):
                    nc.tensor.matmul(
                        ps[:, 0:512],
                        xv_w[ec][:, (lt % 4) * P:(lt % 4 + 1) * P],
                        wq_sb[ec][:, 2 * FG:3 * FG],
                        start=(ec == 0), stop=(ec == EC - 1))
                # strided drains: h0 dims -> cols {0:64}+256p, h1 -> {192:256}+256p
                dst0 = bass.AP(vst[lt].tensor, vst[lt].offset,
                               [[1024, 128], [256, 4], [1, 64]])
                src0 = bass.AP(ps.tensor, ps.offset,
                               [[1536, 128], [128, 4], [1, 64]])
                nc.vector.tensor_copy(dst0, src0)
                dst1 = bass.AP(vst[lt].tensor, vst[lt].offset + 192,
                               [[1024, 128], [256, 4], [1, 64]])
                src1 = bass.AP(ps.tensor, ps.offset + 64,
                               [[1536, 128], [128, 4], [1, 64]])
                nc.vector.tensor_copy(dst1, src1)

            def outproj(lg, lt):
                t0 = lg * 512 + lt * P
                ps = psp.tile([P, 1536], F32, tag="sc", name="ops")
                for ec in range(FT):
                    nc.tensor.matmul(
                        ps[:, 0:512], avN[ec][:, t0:t0 + P],
                        wo_sb[ec][:, 0:512],
                        start=(ec == 0), stop=(ec == FT - 1))
                    nc.tensor.matmul(
                        ps[:, 512:1024], avN[ec][:, t0:t0 + P],
                        wo_sb[ec][:, 512:1024],
                        start=(ec == 0), stop=(ec == FT - 1))
                osb = stage.tile([P, E], F32, tag="osb", name="osb", bufs=2)
                nc.scalar.copy(osb[:], ps[:, 0:1024])
                nc.sync.dma_start(out_d.ap()[t0:t0 + P, :], osb[:])

            pending_tails = []

            def attention_iter(p, lg, hook=None):
                avA = psav.tile([P, 512], F32, tag="avA", name="avA")
                avB = psav.tile([P, 512], F32, tag="avB", name="avB")
                av_bank = (avA, avB)

                def av_mms(t0, n, aT, p=p, av_bank=av_bank):
                    for j in range(n):
                        sc, h = divmod(t0 + j, 2)
                        nc.tensor.matmul(
                            av_bank[h][:],
                            vst[sc][:, p * 256 + 128 * h:p * 256 + 128 * h + 128],
                            aT[:, j * 512:(j + 1) * 512],
                            start=(sc == 0), stop=(sc == SC - 1))

                pending = None
                for gi, t0 in enumerate(range(0, NU, GRP)):
                    n = min(GRP, NU - t0)
                    ps = psp.tile([P, 1536], F32, tag="sc", name="scp")
                    for j in range(n):
                        sc, h = divmod(t0 + j, 2)
                        nc.tensor.matmul(
                            ps[:, j * 512:(j + 1) * 512],
                            kT[p][64 * h:64 * h + 64, sc * P:(sc + 1) * P],
                            qT[p][64 * h:64 * h + 64, lg * 512:(lg + 1) * 512],
                            start=True, stop=True)
                    aT = stage.tile([P, 1536], BF16, tag="aT", name="aT", bufs=3)
                    nc.scalar.activation(aT[:, 0:512 * n], ps[:, 0:512 * n],
                                         EXP, scale=0.125)
                    if debug and p == 0 and lg == 0 and gi == 0:
                        nc.sync.dma_start(dbg_a.ap(), aT[:])
                    if gi == 0 and pending_tails:
                        pending_tails.pop()()  # prev iter's tail after this
                        # iter's first exp is queued
                    if hook is not None:
                        hook(gi)
                    if pending is not None:
                        av_mms(*pending)
                    pending = (t0, n, aT)

                def tail(avA=avA, avB=avB, p=p, lg=lg, pending=pending,
                         av_mms=av_mms):
                    av_mms(*pending)
                    avS0 = stage.tile([P, 512], F32, tag="avS0", name="avS0",
                                      bufs=1)
                    nc.vector.tensor_copy(avS0[:], avA[:])
                    avS1 = stage.tile([P, 512], F32, tag="avS1", name="avS1",
                                      bufs=1)
                    nc.vector.tensor_copy(avS1[:], avB[:])
                    # both denominators on partitions 0/32 -> one reciprocal
                    nc.vector.tensor_copy(rr[0:1, :], avS0[64:65, :])
                    nc.vector.tensor_copy(rr[32:33, :], avS1[32:33, :])
                    nc.vector.reciprocal(rr[0:64, :], rr[0:64, :])
                    r1 = stage.tile([1, 512], F32, tag="r1", name="r1", bufs=1)
                    nc.vector.tensor_copy(r1[0:1, :], rr[32:33, :])
                    bc0 = stage.tile([P, 512], F32, tag="bc0", name="bc0",
                                     bufs=1)
                    nc.gpsimd.partition_broadcast(bc0[:], rr[0:1, :])
                    bc1 = stage.tile([P, 512], F32, tag="bc1", name="bc1",
                                     bufs=1)
                    nc.gpsimd.partition_broadcast(bc1[:], r1[0:1, :])
                    nc.vector.tensor_mul(
                        avN[p][0:64, lg * 512:(lg + 1) * 512],
                        avS0[0:64, :], bc0[0:64, :])
                    nc.vector.tensor_mul(
                        avN[p][64:128, lg * 512:(lg + 1) * 512],
                        avS1[64:128, :], bc1[64:128, :])
                    if debug and p == 0 and lg == 0:
                        nc.sync.dma_start(dbg_s.ap(), avS0[:])
                pending_tails.append(tail)

            # ---- the fused schedule ----
            def vproj_hook(gi):
                if gi < 8:
                    if gi in (2, 4):
                        xvw[gi // 2 + 1] = dma_xv_window(gi // 2 + 1)
                    w = gi // 2
                    vproj(2 * gi, xvw[w])
                    vproj(2 * gi + 1, xvw[w])

            for lg in range(LG):
                for p in range(FT):
                    if lg == 0:
                        kproj(p)
                    qproj(p, lg)
                    attention_iter(p, lg, hook=vproj_hook if (p == 0 and lg == 0)
                                   else None)
                    if lg > 0:
                        outproj(lg - 1, p)
                if lg < LG - 1:
                    xq_w = dma_xq_window(lg + 1)
            while pending_tails:
                pending_tails.pop()()
            for lt in range(FT):
                outproj(LG - 1, lt)
            if debug:
                nc.sync.dma_start(dbg_q.ap(), qT[0][:])
                nc.sync.dma_start(dbg_k.ap(), kT[0][:])
                nc.sync.dma_start(dbg_v.ap(), vst[0][:])
                nc.sync.dma_start(dbg_n.ap(), avN[0][:])

    nc.compile()
    return nc


_NC = None


def _get_nc():
    global _NC
    if _NC is None:
        _NC = _build()
    return _NC


def _shard_inputs(query, key, value, in_proj_weight, out_proj_weight):
    B = query.shape[0]
    # per-batch transposed bf16 activations (shared by the 2 cores per batch)
    xT = {}
    for b in range(B):
        xT[b] = tuple(
            np.ascontiguousarray(np.asarray(x[b], dtype=np.float32).T).astype(NP_BF16)
            for x in (query, key, value))
    # per-head-group weight blocks
    wblk = {}
    for g in range(2):
        sl = slice(FG * g, FG * g + FG)
        wq = in_proj_weight[0 * E:1 * E][sl]
        wk = in_proj_weight[1 * E:2 * E][sl]
        wv = in_proj_weight[2 * E:3 * E][sl]
        wqkvT = np.ascontiguousarray(
            np.concatenate([wq.T, wk.T, wv.T], axis=1)).astype(NP_BF16)
        woutT = np.ascontiguousarray(out_proj_weight[:, sl].T).astype(NP_BF16)
        wblk[g] = (wqkvT, woutT)
    in_maps = []
    for c in range(8):
        b, g = divmod(c, 2)
        xq, xk, xv = xT[b]
        wqkvT, woutT = wblk[g]
        in_maps.append({
            "xqT": xq, "xkT": xk, "xvT": xv,
            "wqkvT": wqkvT, "woutT": woutT,
        })
    return in_maps


def run_sharded(in_maps, **kwargs):
    nc = _get_nc()
    return bass_utils.run_bass_kernel_spmd(
        nc, in_maps, core_ids=list(range(8)), **kwargs)


def kernel(query, key, value, in_proj_weight, out_proj_weight, out_proj_bias):
    query = np.asarray(query, dtype=np.float32)
    key = np.asarray(key, dtype=np.float32)
    value = np.asarray(value, dtype=np.float32)
    in_proj_weight = np.asarray(in_proj_weight, dtype=np.float32)
    out_proj_weight = np.asarray(out_proj_weight, dtype=np.float32)
    out_proj_bias = np.asarray(out_proj_bias, dtype=np.float32)

    in_maps = _shard_inputs(query, key, value, in_proj_weight, out_proj_weight)
    res = run_sharded(in_maps)
    out = np.empty((4, L, E), dtype=np.float32)
    for b in range(4):
        out[b] = res.results[2 * b]["out"] + res.results[2 * b + 1]["out"]
    out += out_proj_bias
    return out


# revision 26
# speedup vs baseline: 1.2149x; 1.0297x over previous
"""Multi-head attention forward (B=4, L=2048, E=1024, H=16) on 8 NeuronCores.

Sharding: core c handles batch b = c // 2 and head-group g = c % 2 (8 heads,
512 embed dims). Each core computes its QKV projections, attention, and a
partial out-projection over its 512 contraction dims; the host sums the two
partials per batch and adds the bias.

All transposes and bf16 casts happen on the host: each core receives
xqT/xkT/xvT as [E, L] bf16, wqkvT as [E, 3*FG] bf16 (cols q|k|v) and
woutT as [FG, E] bf16. On-chip work is a single fused pipeline:
k-proj -> per (p, lg): q-proj, scores (PE), exp (ACT), AV (PE), with
v-proj and the out-projection interleaved into the PE stream so the
tensor engine never waits on the ACT-bound softmax.

Self-contained: only needs numpy + the concourse stack at /opt/trn_rl_repo.
"""

import os
import sys

import numpy as np

sys.path.insert(0, "/opt/trn_rl_repo")

import ml_dtypes  # noqa: E402

import concourse.bass as bass  # noqa: E402
import concourse.tile as tile  # noqa: E402
from concourse import bacc, mybir  # noqa: E402
from concourse import bass_utils  # noqa: E402

F32 = mybir.dt.float32
BF16 = mybir.dt.bfloat16
EXP = mybir.ActivationFunctionType.Exp
NP_BF16 = ml_dtypes.bfloat16

P = 128          # partitions
L = 2048         # sequence length
E = 1024         # embed dim
FG = 512         # per-core feature slice (8 heads x 64)
D = 64           # head dim
EC = E // P      # 8 e-chunks (contraction tiles for projections)
SC = L // P      # 16 s-chunks
LG = L // 512    # 4 q-windows of 512
FT = FG // P     # 4 head pairs
GRP = 3          # score psum banks per exp group
NU = 2 * SC      # 32 (sc, head) units per (p, lg)


def _build():
    nc = bacc.Bacc("TRN2", target_bir_lowering=False, debug=False, num_devices=8)

    debug = bool(os.environ.get("MHA_DEBUG"))
    xqT_d = nc.dram_tensor("xqT", [E, L], BF16, kind="ExternalInput")
    xkT_d = nc.dram_tensor("xkT", [E, L], BF16, kind="ExternalInput")
    xvT_d = nc.dram_tensor("xvT", [E, L], BF16, kind="ExternalInput")
    wqkvT_d = nc.dram_tensor("wqkvT", [E, 3 * FG], BF16, kind="ExternalInput")
    woutT_d = nc.dram_tensor("woutT", [FG, E], BF16, kind="ExternalInput")
    out_d = nc.dram_tensor("out", [L, E], F32, kind="ExternalOutput")
    if debug:
        dbg_q = nc.dram_tensor("dbg_q", [P, L], BF16, kind="ExternalOutput")
        dbg_k = nc.dram_tensor("dbg_k", [P, L], BF16, kind="ExternalOutput")
        dbg_v = nc.dram_tensor("dbg_v", [P, 1024], BF16, kind="ExternalOutput")
        dbg_a = nc.dram_tensor("dbg_a", [P, 1536], BF16, kind="ExternalOutput")
        dbg_s = nc.dram_tensor("dbg_s", [P, 512], F32, kind="ExternalOutput")
        dbg_n = nc.dram_tensor("dbg_n", [P, L], BF16, kind="ExternalOutput")

    with tile.TileContext(nc) as tc:
        with (
            tc.tile_pool(name="const", bufs=1) as constp,
            tc.tile_pool(name="pers", bufs=1) as pers,
            tc.tile_pool(name="xin", bufs=1) as xin,
            tc.tile_pool(name="xv", bufs=2) as xvp,
            tc.tile_pool(name="xq", bufs=2) as xqp,
            tc.tile_pool(name="stage", bufs=2) as stage,
            tc.tile_pool(name="ps", bufs=2, space="PSUM") as psp,
            tc.tile_pool(name="psav", bufs=1, space="PSUM") as psav,
        ):
            # engine warm-ups (prime DVE cast path + preload the EXP table)
            warm32 = constp.tile([P, 16], F32, tag="warm32", name="warm32")
            nc.vector.memset(warm32[:], 0.0)
            warm16 = constp.tile([P, 16], BF16, tag="warm16", name="warm16")
            nc.vector.tensor_copy(warm16[:], warm32[:])
            warmE = constp.tile([P, 16], BF16, tag="warmE", name="warmE")
            nc.scalar.activation(warmE[:], warm32[:], EXP, scale=0.125)
            warmG = constp.tile([P, 16], F32, tag="warmG", name="warmG")
            nc.gpsimd.memset(warmG[:], 0.0)
            # reciprocal staging: rows 0/32 hold denominators per tail, the
            # rest stays 1.0 so the batched [64,512] reciprocal is stable
            rr = constp.tile([P, 512], F32, tag="rr", name="rr")
            nc.vector.memset(rr[:], 1.0)

            # persistent activations / weights
            kT = [pers.tile([P, L], BF16, tag=f"kT{p}", name=f"kT{p}")
                  for p in range(FT)]
            qT = [pers.tile([P, L], BF16, tag=f"qT{p}", name=f"qT{p}")
                  for p in range(FT)]
            avN = [pers.tile([P, L], BF16, tag=f"avN{p}", name=f"avN{p}")
                   for p in range(FT)]
            # AV stationary tiles: per s-chunk, 4 pairs x 256 cols:
            #   [v_h0(64) | ones(1) | junk(63)]  -> av rows 0:64, sum row 64
            #   [junk(32) | ones(1) | junk(31) | v_h1(64)] -> rows 64:128, sum row 32
            # (junk columns only feed av rows that are never read)
            vst = [pers.tile([P, 1024], BF16, tag=f"vst{s}", name=f"vst{s}")
                   for s in range(SC)]
            wq_sb = [pers.tile([P, 3 * FG], BF16, tag=f"wq{ec}", name=f"wq{ec}")
                     for ec in range(EC)]
            wo_sb = [pers.tile([P, E], BF16, tag=f"wo{e}", name=f"wo{e}")
                     for e in range(FT)]

            # ---- input DMAs, ordered by first use (k-proj path first) ----
            for ec in range(EC):
                nc.sync.dma_start(wq_sb[ec][:, FG:2 * FG],
                                  wqkvT_d.ap()[ec * P:(ec + 1) * P, FG:2 * FG])
            xk = [xin.tile([P, L], BF16, tag=f"x{ec}", name=f"xk{ec}")
                  for ec in range(EC)]
            for ec in range(EC):
                nc.sync.dma_start(xk[ec][:], xkT_d.ap()[ec * P:(ec + 1) * P, :])

            # AV stationary pattern: zero the non-v columns, ones at the
            # denominator columns (64, 160 of each 256-block)
            one = int(np.float32(1.0).astype(NP_BF16).view(np.uint16))
            for s in range(SC):
                t = vst[s]
                nc.gpsimd._memset_packed(
                    bass.AP(t.tensor, t.offset + 64,
                            [[1024, 128], [256, 4], [1, 128]]), 0)
                nc.gpsimd._memset_packed(
                    bass.AP(t.tensor, t.offset + 64, [[1024, 128], [256, 4]]),
                    one)
                nc.gpsimd._memset_packed(
                    bass.AP(t.tensor, t.offset + 160, [[1024, 128], [256, 4]]),
                    one)

            def dma_xq_window(lg):
                tiles = [xqp.tile([P, 512], BF16, tag=f"q{ec}", name=f"xq{ec}")
                         for ec in range(EC)]
                for ec in range(EC):
                    nc.sync.dma_start(
                        tiles[ec][:],
                        xqT_d.ap()[ec * P:(ec + 1) * P, lg * 512:(lg + 1) * 512])
                return tiles

            for ec in range(EC):
                nc.sync.dma_start(wq_sb[ec][:, 0:FG],
                                  wqkvT_d.ap()[ec * P:(ec + 1) * P, 0:FG])
            xq_w = dma_xq_window(0)

            def dma_xv_window(w):
                tiles = [xvp.tile([P, 512], BF16, tag=f"v{ec}", name=f"xv{ec}")
                         for ec in range(EC)]
                for ec in range(EC):
                    nc.sync.dma_start(
                        tiles[ec][:],
                        xvT_d.ap()[ec * P:(ec + 1) * P, w * 512:(w + 1) * 512])
                return tiles

            for ec in range(EC):
                nc.sync.dma_start(wq_sb[ec][:, 2 * FG:3 * FG],
                                  wqkvT_d.ap()[ec * P:(ec + 1) * P, 2 * FG:3 * FG])
            xvw = {0: dma_xv_window(0), 1: dma_xv_window(1)}
            for e in range(FT):
                nc.sync.dma_start(wo_sb[e][:], woutT_d.ap()[e * P:(e + 1) * P, :])

            # ---- helpers ----
            def kproj(p):
                # kT[p][:, :] over 4 token windows; 3+1 windows per psum tile
                for w0, nw in ((0, 3), (3, 1)):
                    ps = psp.tile([P, 1536], F32, tag="sc", name="kps")
                    for ec in range(EC):
                        for w in range(nw):
                            nc.tensor.matmul(
                                ps[:, w * 512:(w + 1) * 512],
                                wq_sb[ec][:, FG + p * P:FG + (p + 1) * P],
                                xk[ec][:, (w0 + w) * 512:(w0 + w + 1) * 512],
                                start=(ec == 0), stop=(ec == EC - 1))
                    nc.vector.tensor_copy(
                        kT[p][:, w0 * 512:(w0 + nw) * 512], ps[:, 0:nw * 512])

            def qproj(p, lg):
                ps = psp.tile([P, 1536], F32, tag="sc", name="qps")
                for ec in range(EC):
                    nc.tensor.matmul(
                        ps[:, 0:512],
                        wq_sb[ec][:, p * P:(p + 1) * P],
                        xq_w[ec][:],
                        start=(ec == 0), stop=(ec == EC - 1))
                nc.vector.tensor_copy(
                    qT[p][:, lg * 512:(lg + 1) * 512], ps[:, 0:512])

            def vproj(lt, xv_w):
                ps = psp.tile([P, 1536], F32, tag="sc", name="vps")
                for ec in range(EC):
                    nc.tensor.matmul(
                        ps[:, 0:512],
                        xv_w[ec][:, (lt % 4) * P:(lt % 4 + 1) * P],
                        wq_sb[ec][:, 2 * FG:3 * FG],
                        start=(ec == 0), stop=(ec == EC - 1))
                # strided drains: h0 dims -> cols {0:64}+256p, h1 -> {192:256}+256p
                dst0 = bass.AP(vst[lt].tensor, vst[lt].offset,
                               [[1024, 128], [256, 4], [1, 64]])
                src0 = bass.AP(ps.tensor, ps.offset,
                               [[1536, 128], [128, 4], [1, 64]])
                nc.vector.tensor_copy(dst0, src0)
                dst1 = bass.AP(vst[lt].tensor, vst[lt].offset + 192,
                               [[1024, 128], [256, 4], [1, 64]])
                src1 = bass.AP(ps.tensor, ps.offset + 64,
                               [[1536, 128], [128, 4], [1, 64]])
                nc.vector.tensor_copy(dst1, src1)

            def outproj(lg, lt):
                t0 = lg * 512 + lt * P
                ps = psp.tile([P, 1536], F32, tag="sc", name="ops")
                for ec in range(FT):
                    nc.tensor.matmul(
                        ps[:, 0:512], avN[ec][:, t0:t0 + P],
                        wo_sb[ec][:, 0:512],
                        start=(ec == 0), stop=(ec == FT - 1))
                    nc.tensor.matmul(
                        ps[:, 512:1024], avN[ec][:, t0:t0 + P],
                        wo_sb[ec][:, 512:1024],
                        start=(ec == 0), stop=(ec == FT - 1))
                osb = stage.tile([P, E], F32, tag="osb", name="osb", bufs=2)
                nc.scalar.copy(osb[:], ps[:, 0:1024])
                nc.sync.dma_start(out_d.ap()[t0:t0 + P, :], osb[:])

            pending_tails = []

            def attention_iter(p, lg, hook=None):
                avA = psav.tile([P, 512], F32, tag="avA", name="avA")
                avB = psav.tile([P, 512], F32, tag="avB", name="avB")
                av_bank = (avA, avB)

                def av_mms(t0, n, aT, p=p, av_bank=av_bank):
                    for j in range(n):
                        sc, h = divmod(t0 + j, 2)
                        nc.tensor.matmul(
                            av_bank[h][:],
                            vst[sc][:, p * 256 + 128 * h:p * 256 + 128 * h + 128],
                            aT[:, j * 512:(j + 1) * 512],
                            start=(sc == 0), stop=(sc == SC - 1))

                pending = None
                for gi, t0 in enumerate(range(0, NU, GRP)):
                    n = min(GRP, NU - t0)
                    ps = psp.tile([P, 1536], F32, tag="sc", name="scp")
                    for j in range(n):
                        sc, h = divmod(t0 + j, 2)
                        nc.tensor.matmul(
                            ps[:, j * 512:(j + 1) * 512],
                            kT[p][64 * h:64 * h + 64, sc * P:(sc + 1) * P],
                            qT[p][64 * h:64 * h + 64, lg * 512:(lg + 1) * 512],
                            start=True, stop=True)
                    aT = stage.tile([P, 1536], BF16, tag="aT", name="aT", bufs=3)
                    nc.scalar.activation(aT[:, 0:512 * n], ps[:, 0:512 * n],
                                         EXP, scale=0.125)
                    if debug and p == 0 and lg == 0 and gi == 0:
                        nc.sync.dma_start(dbg_a.ap(), aT[:])
                    if gi == 0 and pending_tails:
                        pending_tails.pop()()  # prev iter's tail after this
                        # iter's first exp is queued
                    if hook is not None:
                        hook(gi)
                    if pending is not None:
                        av_mms(*pending)
                    pending = (t0, n, aT)

                def tail(avA=avA, avB=avB, p=p, lg=lg, pending=pending,
                         av_mms=av_mms):
                    av_mms(*pending)
                    avS0 = stage.tile([P, 512], F32, tag="avS0", name="avS0",
                                      bufs=1)
                    nc.vector.tensor_copy(avS0[:], avA[:])
                    avS1 = stage.tile([P, 512], F32, tag="avS1", name="avS1",
                                      bufs=1)
                    nc.vector.tensor_copy(avS1[:], avB[:])
                    # both denominators on partitions 0/32 -> one reciprocal
                    nc.vector.tensor_copy(rr[0:1, :], avS0[64:65, :])
                    nc.vector.tensor_copy(rr[32:33, :], avS1[32:33, :])
                    nc.vector.reciprocal(rr[0:64, :], rr[0:64, :])
                    r1 = stage.tile([1, 512], F32, tag="r1", name="r1", bufs=1)
                    nc.vector.tensor_copy(r1[0:1, :], rr[32:33, :])
                    bc0 = stage.tile([P, 512], F32, tag="bc0", name="bc0",
                                     bufs=1)
                    nc.gpsimd.partition_broadcast(bc0[:], rr[0:1, :])
                    bc1 = stage.tile([P, 512], F32, tag="bc1", name="bc1",
                                     bufs=1)
                    nc.gpsimd.partition_broadcast(bc1[:], r1[0:1, :])
                    nc.vector.tensor_mul(
                        avN[p][0:64, lg * 512:(lg + 1) * 512],
                        avS0[0:64, :], bc0[0:64, :])
                    nc.vector.tensor_mul(
                        avN[p][64:128, lg * 512:(lg + 1) * 512],
                        avS1[64:128, :], bc1[64:128, :])
                    if debug and p == 0 and lg == 0:
                        nc.sync.dma_start(dbg_s.ap(), avS0[:])
                pending_tails.append(tail)

            # ---- the fused schedule ----
            def vproj_hook(gi):
                if gi < 8:
                    if gi in (2, 4):
                        xvw[gi // 2 + 1] = dma_xv_window(gi // 2 + 1)
                    w = gi // 2
                    vproj(2 * gi, xvw[w])
                    vproj(2 * gi + 1, xvw[w])

            for p in range(FT):
                kproj(p)
            for lg in range(LG):
                for p in range(FT):
                    qproj(p, lg)
                    attention_iter(p, lg, hook=vproj_hook if (p == 0 and lg == 0)
                                   else None)
                    if lg > 0:
                        outproj(lg - 1, p)
                if lg < LG - 1:
                    xq_w = dma_xq_window(lg + 1)
            while pending_tails:
                pending_tails.pop()()
            for lt in range(FT):
                outproj(LG - 1, lt)
            if debug:
                nc.sync.dma_start(dbg_q.ap(), qT[0][:])
                nc.sync.dma_start(dbg_k.ap(), kT[0][:])
                nc.sync.dma_start(dbg_v.ap(), vst[0][:])
                nc.sync.dma_start(dbg_n.ap(), avN[0][:])

    nc.compile()
    return nc


_NC = None


def _get_nc():
    global _NC
    if _NC is None:
        _NC = _build()
    return _NC


def _shard_inputs(query, key, value, in_proj_weight, out_proj_weight):
    B = query.shape[0]
    # per-batch transposed bf16 activations (shared by the 2 cores per batch)
    xT = {}
    for b in range(B):
        xT[b] = tuple(
            np.ascontiguousarray(np.asarray(x[b], dtype=np.float32).T).astype(NP_BF16)
            for x in (query, key, value))
    # per-head-group weight blocks
    wblk = {}
    for g in range(2):
        sl = slice(FG * g, FG * g + FG)
        wq = in_proj_weight[0 * E:1 * E][sl]
        wk = in_proj_weight[1 * E:2 * E][sl]
        wv = in_proj_weight[2 * E:3 * E][sl]
        wqkvT = np.ascontiguousarray(
            np.concatenate([wq.T, wk.T, wv.T], axis=1)).astype(NP_BF16)
        woutT = np.ascontiguousarray(out_proj_weight[:, sl].T).astype(NP_BF16)
        wblk[g] = (wqkvT, woutT)
    in_maps = []
    for c in range(8):
        b, g = divmod(c, 2)
        xq, xk, xv = xT[b]
        wqkvT, woutT = wblk[g]
        in_maps.append({
            "xqT": xq, "xkT": xk, "xvT": xv,
            "wqkvT": wqkvT, "woutT": woutT,
        })
    return in_maps


def run_sharded(in_maps, **kwargs):
    nc = _get_nc()
    return bass_utils.run_bass_kernel_spmd(
        nc, in_maps, core_ids=list(range(8)), **kwargs)


def kernel(query, key, value, in_proj_weight, out_proj_weight, out_proj_bias):
    query = np.asarray(query, dtype=np.float32)
    key = np.asarray(key, dtype=np.float32)
    value = np.asarray(value, dtype=np.float32)
    in_proj_weight = np.asarray(in_proj_weight, dtype=np.float32)
    out_proj_weight = np.asarray(out_proj_weight, dtype=np.float32)
    out_proj_bias = np.asarray(out_proj_bias, dtype=np.float32)

    in_maps = _shard_inputs(query, key, value, in_proj_weight, out_proj_weight)
    res = run_sharded(in_maps)
    out = np.empty((4, L, E), dtype=np.float32)
    for b in range(4):
        out[b] = res.results[2 * b]["out"] + res.results[2 * b + 1]["out"]
    out += out_proj_bias
    return out
